# revision 13
# baseline (speedup 1.0000x reference)
"""Trainium2 Bass kernel v2: 14-qubit data-reuploading quantum circuit actor.

Core idea vs v1: hand-authored 2x_1p custom-DVE *pair* ops on interleaved
complex fp16 — lo/hi lanes of the packed-fp16 datapath compute (re, im) of a
complex multiply, so a full merged per-wire gate U = RY(v)RZ(b)RY(a) is 4 fat
instructions at 2 elems/cycle:

    T   = U00 (x) X        (CMULIGN:  out = (C0+iC1) (x) in0)
    B.X = U01 (x) Y + T    (CMULACC:  out = (C0+iC1) (x) in0 + in1)
    T   = U10 (x) X
    B.Y = U11 (x) Y + T

U structure (alpha = RY input half-angle, beta = RZ half-angle incl weight-RZ,
vh = weight-RY half-angle): with p = alpha+vh, m = alpha-vh:
    A1 = cos(beta) cos(p); A2 = sin(beta) cos(m)
    B1 = cos(beta) sin(p); B2 = sin(beta) sin(m)
    U00 = A1 - i A2 ; U01 = -B1 + i B2 ; U10 = B1 + i B2 ; U11 = A1 + i A2

CNOT(t-1, t) of the ring folds into wire-t's write APs (region split on bit
t-1); CNOT(13, 0) folds into the next layer's wire-0 reads (and into the
measurement reads for the last layer). perf_max=1 is stamped on each pair-op
instruction post-Tile so the RTL engages the 2x_1p uop slot (validated on HW:
the 2x program's pair semantics only appear with perf_max=1).

Layer 1 acts on |0..0>, so it is replaced by a product-state doubling build
(~54 tiny ops instead of 14 full-state gates), with the ring folded into the
append APs.

Inputs shipped per call: ONE packed f32 array per core (256 rows of x +
20 rows holding the 280-float aux table of host-precomputed isc/2, weights/2
terms), ~124KB total vs 1.7MB for the v1 angle table. Angles + trig
(range-wrap + Horner minimax) + the 6 coefficient planes are computed
on-chip per 128-row tile.

Measured: CoreSim exec 4.11 ms/core (v1 baseline: 19.0 ms), of which gates
are 94.4% running at 98.5% of the 2-elem/cycle DVE stream rate (and within
~12% of the 8-ALU-stage pipeline floor: 28 ALU ops per amplitude pair, 16
distinct scalar-x-element products, no sharing possible — verified). HW warm
wall 54-70 ms/call (load-dependent), ~50-60 ms of which is the fixed
axon-tunnel dispatch+fetch round trip (invariant to core count, bytes, or
work). Call path: fresh async device_put of the packed input each call
(reusing a committed device array is ~25 ms slower through axon) + donated
zero output buffers re-staged on device between calls.
Relative rms error vs reference: 3.0e-3 (v1: 4.0e-3; gate: 2e-2).
"""

import numpy as np

NQ = 14
NL = 5
OBS = 14
NA = 6
B = 2048
NCORES = 8
BPC = B // NCORES          # 256 rows per core
PT = 128                   # partitions per tile
NTILES = BPC // PT         # 2
NS = 1 << NQ               # 16384 amplitudes
F = 2 * NS                 # 32768 floats per row (interleaved complex)
NW = NL * NQ               # 70 (layer, wire) pairs
PI = float(np.pi)

# aux layout (floats, [1, 4*NW]): ISC1 | ISC2 | WT1 | VH
#   ISC1[l,w] = input_scaling[l,w]/2
#   ISC2[l,w] = input_scaling[l,w+14]/2
#   WT1[l,w]  = weights[l,w]/2
#   VH[l,w]   = weights[l,w+14]/2
NAUX = 4 * NW

SIN_P = [2.2248706406891887e-06, -0.00019424154210166545,
         0.008319842398281522, -0.16665145941120196,
         0.9999972898367918]
COS_Q = [-2.219394993734796e-07, 2.42531924958235e-05,
         -0.001386274731586208, 0.04166103279007339,
         -0.4999955816555398, 0.9999994436793969]


def make_aux(input_scaling, weights):
    isc = np.asarray(input_scaling, np.float64)
    wt = np.asarray(weights, np.float64)
    aux = np.concatenate([
        (isc[:, :NQ] / 2.0).ravel(),
        (isc[:, NQ:] / 2.0).ravel(),
        (wt[:, :NQ] / 2.0).ravel(),
        (wt[:, NQ:] / 2.0).ravel(),
    ]).astype(np.float32)
    return aux.reshape(1, NAUX)


def coef_planes(x, aux):
    """Host/numpy mirror of the on-chip coef computation (float64 path).
    x: (n, 14) -> dict of (n, 70) planes A1, A2, B1, B2."""
    x = np.asarray(x, np.float64)
    a = np.asarray(aux, np.float64).ravel()
    isc1 = a[0:NW].reshape(NL, NQ)
    isc2 = a[NW:2 * NW].reshape(NL, NQ)
    wt1 = a[2 * NW:3 * NW].reshape(NL, NQ)
    vh = a[3 * NW:4 * NW].reshape(NL, NQ)
    xb = x[:, None, :]
    alpha = isc1[None] * xb
    beta = isc2[None] * xb + wt1[None]
    p = alpha + vh[None]
    m = alpha - vh[None]
    cb, sb = np.cos(beta), np.sin(beta)
    return {
        "A1": (cb * np.cos(p)).reshape(-1, NW).astype(np.float32),
        "A2": (sb * np.cos(m)).reshape(-1, NW).astype(np.float32),
        "B1": (cb * np.sin(p)).reshape(-1, NW).astype(np.float32),
        "B2": (sb * np.sin(m)).reshape(-1, NW).astype(np.float32),
    }


# ---------------------------------------------------------------- schedule
# region = (buf, float_offset, dims); dims = ((step, count), ...) innermost
# last, float-index space. buf: "A"/"B" state, "T" scratch (16384 floats),
# "S" s64 sums. Every pair-op region: innermost step 1, even count, even
# offset (2x_1p eligibility).
#
# ops:
#  ("cmulign", dst, src, c0, c1): dst = (c0+ic1) (x) src      [in1 ignored]
#  ("cmulacc", dst, src, acc, c0, c1): dst = (c0+ic1)(x)src + acc (acc rank-1)
#  ("sqsum", dst, s0, s1): dst = s0^2 + s1^2
#  ("red", dst, src): 64-block reduce
# scalar ref = (plane, col), plane in A1,A2,B1,B2,NA2,NB1.


def _norm(dims):
    d = [(s, c) for s, c in dims if c != 1]
    out = []
    for s, c in d:
        if out and out[-1][0] == s * c:
            out[-1] = (s, c * out[-1][1])
        else:
            out.append((s, c))
    if not out:
        out = [(1, 1)]
    assert len(out) <= 2, out
    return tuple(out)


def _reg(buf, off, *dims):
    return (buf, off, _norm(dims))


def _nelem(reg):
    n = 1
    for _, c in reg[2]:
        n *= c
    return n


class Sched:
    def __init__(self):
        self.ops = []
        self.cur = "A"

    def swap(self):
        self.cur = "B" if self.cur == "A" else "A"

    def gate(self, l, t):
        """Merged U(l, t) with ring-fold on writes (t>=1) and prev-layer
        C(13,0) fold on reads (t==0, l>=1)."""
        a, b = self.cur, "B" if self.cur == "A" else "A"
        col = l * NQ + t
        A1, A2 = ("A1", col), ("A2", col)
        B1, B2 = ("B1", col), ("B2", col)
        NA2, NB1 = ("NA2", col), ("NB1", col)

        if t == 0:
            if l == 0:
                X = [_reg(a, 0, (1, NS))]
                Y = [_reg(a, NS, (1, NS))]
                DX = [_reg(b, 0, (1, NS))]
                DY = [_reg(b, NS, (1, NS))]
            else:
                d = ((4, NS // 4), (1, 2))
                X = [_reg(a, 0, *d), _reg(a, NS + 2, *d)]
                Y = [_reg(a, NS, *d), _reg(a, 2, *d)]
                DX = [_reg(b, 0, *d), _reg(b, 2, *d)]
                DY = [_reg(b, NS, *d), _reg(b, NS + 2, *d)]
        else:
            Ft = 1 << (14 - t)
            nb = 1 << (t - 1)
            d = ((4 * Ft, nb), (1, Ft))
            X = [_reg(a, 0, *d), _reg(a, 2 * Ft, *d)]
            Y = [_reg(a, Ft, *d), _reg(a, 3 * Ft, *d)]
            # ring C(t-1, t): odd-b (bit t-1 = 1) writes land bit-t-flipped
            DX = [_reg(b, 0, *d), _reg(b, 3 * Ft, *d)]    # out0 -> X | Yo
            DY = [_reg(b, Ft, *d), _reg(b, 2 * Ft, *d)]   # out1 -> Y | Xo

        nparts = len(X)
        half = NS if nparts == 1 else NS // 2
        for i in range(nparts):
            Ti = _reg("T", i * half, (1, half))
            self.ops.append(("cmulign", Ti, X[i], A1, NA2))        # U00 (x) X
            self.ops.append(("cmulacc", DX[i], Y[i], Ti, NB1, B2))  # +U01 (x) Y
        for i in range(nparts):
            Ti = _reg("T", i * half, (1, half))
            self.ops.append(("cmulign", Ti, X[i], B1, B2))          # U10 (x) X
            self.ops.append(("cmulacc", DY[i], Y[i], Ti, A1, A2))   # +U11 (x) Y
        self.swap()

    def layer0_build(self):
        """Layer-1 on |0..0>: product state via doubling, appending qubit w
        as the new innermost index; ring C(w-1, w) folds into the append APs
        (odd source index j <-> bit w-1 = 1 -> flip new bit w).
        s_0..s_11 ping-pong in T halves, s_12 -> B[0:16384], s_13 -> A."""
        assert self.cur == "A"
        ops = self.ops
        # seed: T[0:2] = (1, 0) — emitted by the bass builder (memset), and
        # by the numpy executor, via the special op below.
        ops.append(("seed",))

        def v0(w):
            return ("A1", w), ("NA2", w)   # U00 column entry

        def v1(w):
            return ("B1", w), ("B2", w)    # U10

        def place(k):
            # buffer holding s_k (size 2**(k+2) floats)
            if k <= 11:
                return ("T", 8192 * (k % 2))
            if k == 12:
                return ("B", 0)
            return ("A", 0)

        # qubit 0: s_0 from seed (no fold)
        c0, s0 = v0(0)
        c1, s1 = v1(0)
        seed = _reg("T", 0, (1, 2))
        dstb, dsto = place(0)
        # b=1 first (disjoint), then b=0 in-place over the seed
        ops.append(("cmulign", _reg(dstb, dsto + 2, (4, 1), (1, 2)), seed, c1, s1))
        ops.append(("cmulign", _reg(dstb, dsto + 0, (4, 1), (1, 2)), seed, c0, s0))
        for w in range(1, NQ):
            sb, so = place(w - 1)
            db, do = place(w)
            nE = 1 << (w - 1)  # even-j count == odd-j count
            srcE = _reg(sb, so + 0, (4, nE), (1, 2))
            srcO = _reg(sb, so + 2, (4, nE), (1, 2))
            c0, s0 = v0(w)
            c1, s1 = v1(w)
            # b=0 (U00 factor): even j -> 2j ; odd j -> 2j+1 (bit-w flip)
            ops.append(("cmulign", _reg(db, do + 0, (8, nE), (1, 2)), srcE, c0, s0))
            ops.append(("cmulign", _reg(db, do + 6, (8, nE), (1, 2)), srcO, c0, s0))
            # b=1 (U10 factor): even j -> 2j+1 ; odd j -> 2j
            ops.append(("cmulign", _reg(db, do + 2, (8, nE), (1, 2)), srcE, c1, s1))
            ops.append(("cmulign", _reg(db, do + 4, (8, nE), (1, 2)), srcO, c1, s1))
        # s_13 landed in A; cur stays "A"

    def measurement(self):
        a = self.cur
        self.ops.append(("sqsum", _reg("T", 0, (2, 8192)),
                         _reg(a, 0, (4, 8192)), _reg(a, 1, (4, 8192))))
        self.ops.append(("sqsum", _reg("T", 1, (2, 4096)),
                         _reg(a, NS + 2, (4, 4096)), _reg(a, NS + 3, (4, 4096))))
        self.ops.append(("sqsum", _reg("T", 8193, (2, 4096)),
                         _reg(a, 2, (4, 4096)), _reg(a, 3, (4, 4096))))
        self.ops.append(("red", ("S", 0, ((1, 64),)),
                         ("T", 0, ((256, 64), (1, 256)))))


def build_schedule():
    S = Sched()
    S.layer0_build()
    for l in range(1, NL):
        for t in range(NQ):
            S.gate(l, t)
    S.measurement()
    return S.ops


# ------------------------------------------------------------ numpy executor


def _indices(reg):
    _, off, dims = reg
    idx = np.array([0], np.int64)
    for st, ct in dims:
        idx = (idx[:, None] + (np.arange(ct, dtype=np.int64) * st)[None, :]).ravel()
    return off + idx


def simulate_numpy(x, aux, fp16=True):
    """x: (n, 14) -> (n, 64) block sums, mirroring the on-device schedule."""
    pl = coef_planes(x, aux)
    pl = dict(pl)
    pl["NA2"] = -pl["A2"]
    pl["NB1"] = -pl["B1"]
    n = x.shape[0]
    sdt = np.float16 if fp16 else np.float32
    bufs = {
        "A": np.zeros((n, F), sdt),
        "B": np.zeros((n, F), sdt),
        "T": np.zeros((n, NS), sdt),
        "S": np.zeros((n, 64), np.float32),
    }
    bufs["A"][:, 0] = 1.0

    def cmul(src_v, c, s):
        lo, hi = src_v[:, 0::2], src_v[:, 1::2]
        out = np.empty_like(src_v)
        out[:, 0::2] = c * lo - s * hi
        out[:, 1::2] = s * lo + c * hi
        return out

    for op in build_schedule():
        kind = op[0]
        if kind == "seed":
            bufs["T"][:, 0] = 1.0
            bufs["T"][:, 1] = 0.0
        elif kind in ("cmulign", "cmulacc"):
            if kind == "cmulign":
                _, dst, src, c0, c1 = op
                acc_v = 0.0
            else:
                _, dst, src, acc, c0, c1 = op
                acc_v = bufs[acc[0]][:, _indices(acc)].astype(np.float32)
            c = pl[c0[0]][:n, c0[1]:c0[1] + 1].astype(np.float32)
            s = pl[c1[0]][:n, c1[1]:c1[1] + 1].astype(np.float32)
            src_v = bufs[src[0]][:, _indices(src)].astype(np.float32)
            v = cmul(src_v, c, s) + acc_v
            bufs[dst[0]][:, _indices(dst)] = v.astype(sdt)
        elif kind == "sqsum":
            _, dst, s0, s1 = op
            v = (bufs[s0[0]][:, _indices(s0)].astype(np.float32) ** 2
                 + bufs[s1[0]][:, _indices(s1)].astype(np.float32) ** 2)
            bufs[dst[0]][:, _indices(dst)] = v.astype(sdt)
        elif kind == "red":
            _, dst, src = op
            v = bufs[src[0]][:, _indices(src)].astype(np.float32)
            bufs["S"][:, _indices(dst)] = v.reshape(n, 64, 256).sum(axis=2)
        else:
            raise ValueError(kind)
    return bufs["S"].copy()


def postprocess(s64, action_scale, action_bias):
    blk = np.arange(64)
    out = np.zeros((s64.shape[0], NA), np.float32)
    for w in range(NA):
        sign = 1.0 - 2.0 * ((blk >> (5 - w)) & 1)
        out[:, w] = s64 @ sign.astype(np.float32)
    return out * np.asarray(action_scale, np.float32) + np.asarray(
        action_bias, np.float32)


# ------------------------------------------------------------------ DVE ops

_CUSTOM = {}


def _build_pair_uop(with_acc):
    from concourse.dve_uop import (
        InpSel, OutSel, AluInp as D, DelayInp, OutPath, Trigger, UopConfig,
        UopDpConfig, AluOp, ENABLE)

    def dp(op, a, b, capture=None, passes=()):
        d = UopDpConfig().enable_alu(op, a, b)
        if capture is not None:
            d.enable_delay_from_src(DelayInp.PREV_ALU_OUT, capture)
        if passes:
            d.pass_through_delay(*passes)
        return d

    u = UopConfig()
    u.enable_input(InpSel.SRC_0, 1)      # d0 = X_lo
    u.enable_input(InpSel.CONST_0, 2)    # d1 = C0
    u.enable_input(InpSel.SRC_0_HI, 3)   # d2 = X_hi
    u.enable_input(InpSel.CONST_1, 4)    # d3 = C1
    u.enable_input(InpSel.SRC_1, 5)      # d4 = T_lo
    u.enable_input(InpSel.SRC_1_HI, 6)   # d5 = T_hi
    if with_acc:
        u.datapath_config[0] = dp(AluOp.MULTIPLY, D.PREV_DELAY_0, D.PREV_DELAY_1,
                                  passes=(0, 1, 2, 3, 4, 5))
        u.datapath_config[1] = dp(AluOp.ADD, D.PREV_ALU_OUT, D.PREV_DELAY_4,
                                  passes=(0, 1, 2, 3, 5))
        u.datapath_config[2] = dp(AluOp.MULTIPLY, D.PREV_DELAY_2, D.PREV_DELAY_3,
                                  capture=4, passes=(0, 1, 2, 3, 5))
        u.datapath_config[3] = dp(AluOp.SUBTRACT, D.PREV_DELAY_4, D.PREV_ALU_OUT,
                                  passes=(0, 1, 2, 3, 5))
        u.datapath_config[4] = dp(AluOp.MULTIPLY, D.PREV_DELAY_0, D.PREV_DELAY_3,
                                  capture=4, passes=(1, 2, 5))
        u.datapath_config[5] = dp(AluOp.MULTIPLY, D.PREV_DELAY_2, D.PREV_DELAY_1,
                                  capture=0, passes=(4, 5))
        u.datapath_config[6] = dp(AluOp.ADD, D.PREV_DELAY_0, D.PREV_ALU_OUT,
                                  passes=(4, 5))
        u.datapath_config[7] = dp(AluOp.ADD, D.PREV_ALU_OUT, D.PREV_DELAY_5,
                                  passes=(4,))
    else:
        u.datapath_config[0] = dp(AluOp.MULTIPLY, D.PREV_DELAY_0, D.PREV_DELAY_1,
                                  passes=(0, 1, 2, 3))
        u.datapath_config[1] = dp(AluOp.MULTIPLY, D.PREV_DELAY_2, D.PREV_DELAY_3,
                                  capture=4, passes=(0, 1, 2, 3))
        u.datapath_config[2] = dp(AluOp.SUBTRACT, D.PREV_DELAY_4, D.PREV_ALU_OUT,
                                  passes=(0, 1, 2, 3))
        u.datapath_config[3] = dp(AluOp.MULTIPLY, D.PREV_DELAY_0, D.PREV_DELAY_3,
                                  capture=4, passes=(1, 2))
        u.datapath_config[4] = dp(AluOp.MULTIPLY, D.PREV_DELAY_2, D.PREV_DELAY_1,
                                  capture=5, passes=(4,))
        u.datapath_config[5] = dp(AluOp.ADD, D.PREV_DELAY_5, D.PREV_ALU_OUT,
                                  passes=(4,))
        u.datapath_config[6] = dp(AluOp.BYPASS, D.PREV_ALU_OUT, D.PREV_ALU_OUT,
                                  passes=(4,))
        u.datapath_config[7] = dp(AluOp.BYPASS, D.PREV_ALU_OUT, D.PREV_ALU_OUT,
                                  passes=(4,))
    u.enable_output(OutSel.DELAY_4, OutPath.WR0_LO)
    u.enable_output(OutSel.ALU_OUT, OutPath.WR0_HI)
    u.require_inp0 = ENABLE
    u.require_inp1 = ENABLE
    u.trigger = (Trigger.SRC_TENSOR_DONE, Trigger.NONE, Trigger.NONE)
    u.next_uop = (0, 0, 0)
    return u


def _sc_np(s, p):
    s = np.asarray(s, np.float32)
    return s.reshape(p, -1) if s.size > 1 else s.reshape(-1)


def _cmulacc_ref(in0, in1, s0, s1, imm2):
    p = in0.shape[0]
    x = np.asarray(in0, np.float32).reshape(p, -1)
    t = np.asarray(in1, np.float32).reshape(p, -1)
    c, s = _sc_np(s0, p), _sc_np(s1, p)
    out = np.empty_like(x)
    out[:, 0::2] = c * x[:, 0::2] - s * x[:, 1::2] + t[:, 0::2]
    out[:, 1::2] = s * x[:, 0::2] + c * x[:, 1::2] + t[:, 1::2]
    return out.reshape(in0.shape)


def _cmulign_ref(in0, in1, s0, s1, imm2):
    p = in0.shape[0]
    x = np.asarray(in0, np.float32).reshape(p, -1)
    c, s = _sc_np(s0, p), _sc_np(s1, p)
    out = np.empty_like(x)
    out[:, 0::2] = c * x[:, 0::2] - s * x[:, 1::2]
    out[:, 1::2] = s * x[:, 0::2] + c * x[:, 1::2]
    return out.reshape(in0.shape)


def _sqsum_ref(in0, in1, s0, s1, imm2):
    p = in0.shape[0]
    a = np.asarray(in0, np.float32).reshape(p, -1)
    b = np.asarray(in1, np.float32).reshape(p, -1)
    return (a * a + b * b).reshape(in0.shape)


def _get_custom_ops():
    if _CUSTOM:
        return _CUSTOM
    from concourse import dve_ops
    from concourse.dve_ops import DveOp, OPS
    from concourse.dve_spec import Spec, Src0, Src1, C0, C1, sq, lower
    from concourse.dve_uop import DveOpSpec

    _SPEC_CACHE = {}

    def register(name, body, ref, uop2x):
        for op in OPS:
            if op.name == name:
                return op
        row = dve_ops._CUSTOM_DVE_ROW_BASE + len(OPS)
        spec = Spec(body=body, reference=ref)

        if uop2x is not None:
            class DveOpPair(DveOp):
                def compile(self, ver):
                    key = (self.name, ver)
                    if key in _SPEC_CACHE:
                        return _SPEC_CACHE[key]
                    s = DveOpSpec(
                        name=self.name,
                        opcode=dve_ops.get_dve_sub_opcode(self.name),
                        uops=lower(self.spec, ver=ver),
                        uops_2x=[uop2x],
                        perf_max=1,
                        rd1_en=True,
                    )
                    got = s.sha(ver)
                    if self.uops_sha.get(ver) != got:
                        raise ValueError(f"{self.name}: sha drift {got}")
                    _SPEC_CACHE[key] = s
                    return s
            cls = DveOpPair
        else:
            cls = DveOp
        shas = {}
        for ver in ("v3", "v4"):
            kw = dict(uops_2x=[uop2x], perf_max=1) if uop2x is not None else {}
            s = DveOpSpec(name=name, opcode=row, uops=lower(spec, ver=ver),
                          rd1_en=True, **kw)
            shas[ver] = s.sha(ver)
        op = cls(name, spec, subdim=False, uops_sha=shas)
        OPS.append(op)
        dve_ops._SUB_OPCODE_FOR_NAME[name] = row
        dve_ops.CUSTOM_DVE_SPECS[name] = spec
        return op

    # 1x placeholder bodies are flat (wrong for pair semantics) — correctness
    # depends on the 2x slot engaging; emitter asserts AP eligibility.
    _CUSTOM["cmulacc"] = register(
        "CMULACC_K", Src0 * C0 + Src1 * C1, _cmulacc_ref, _build_pair_uop(True))
    _CUSTOM["cmulign"] = register(
        "CMULIGN_K", Src0 * C0 + Src1 * C1, _cmulign_ref, _build_pair_uop(False))
    _CUSTOM["sqsum"] = register(
        "SQSUM_K", sq(Src0) + sq(Src1), _sqsum_ref, None)
    return _CUSTOM


# ------------------------------------------------------------------ bass side


def _ap(bass_mod, tile_ap, reg):
    t = tile_ap.tensor
    part = list(tile_ap.ap)[0]
    dims = [[part[0], part[1]]] + [[s, c] for s, c in reg[2]]
    return bass_mod.AP(t, tile_ap.offset + reg[1], dims)


def _check_pair_eligible(reg):
    _, off, dims = reg
    assert off % 2 == 0, reg
    st, ct = dims[-1]
    assert st == 1 and ct >= 2 and ct % 2 == 0, reg
    if len(dims) == 2:
        assert dims[0][0] % 2 == 0, reg


def build_bass():
    import concourse.bass as bass
    import concourse.mybir as mybir
    import concourse.tile as tile
    from concourse import bacc
    from contextlib import ExitStack

    f32 = mybir.dt.float32
    f16 = mybir.dt.float16
    cops = _get_custom_ops()
    sched = build_schedule()
    mul_op, add_op = mybir.AluOpType.mult, mybir.AluOpType.add

    nc = bacc.Bacc("TRN2", target_bir_lowering=False, debug=False)
    # packed input: rows 0..BPC-1 = x, rows BPC..BPC+19 = aux (NAUX=280
    # floats as 20 rows of 14); one array per call = one transfer
    xa_d = nc.dram_tensor("xa", [BPC + NAUX // OBS, OBS], f32,
                          kind="ExternalInput").ap()
    out_d = nc.dram_tensor("out", [BPC, NA], f32, kind="ExternalOutput").ap()

    pm_names = []

    def emit_pair(kind, dst_ap, src_ap, in1_ap, s0, s1):
        inst = nc.vector._custom_dve(
            cops[kind], out=dst_ap, in0=src_ap, in1=in1_ap, s0=s0, s1=s1)
        raw = inst.ins if hasattr(inst, "ins") else inst
        pm_names.append(raw.name)
        return inst

    with tile.TileContext(nc) as tc, ExitStack() as ctx:
        state_p = ctx.enter_context(tc.tile_pool(name="state", bufs=1))
        io_p = ctx.enter_context(tc.tile_pool(name="io", bufs=2))

        A_t = state_p.tile([PT, F], f16, tag="A")
        B_t = state_p.tile([PT, F], f16, tag="B")
        T_t = state_p.tile([PT, NS], f16, tag="T")
        aux_t = state_p.tile([PT, NAUX], f32, tag="aux")
        ANG_t = state_p.tile([PT, 6 * NW], f32, tag="ang")  # y(210) | t2(210)
        CS_t = state_p.tile([PT, 6 * NW], f32, tag="cs")   # cos(210) | sin(210)
        PL_t = state_p.tile([PT, 6 * NW], f32, tag="pl")   # A1 A2 B1 B2 NA2 NB1
        W_t = state_p.tile([PT, 3 * NW], f32, tag="w")     # raw angles p|m|beta
        sg_t = state_p.tile([PT, 6 * 64], f32, tag="sg")
        s64_t = state_p.tile([PT, 64], f32, tag="s64")
        r64_t = state_p.tile([PT, 64], f32, tag="r64")

        # aux broadcast to all partitions: 1 DMA + 7 doubling DMAs
        nc.sync.dma_start(
            aux_t[0:1, :],
            bass.AP(xa_d.tensor, xa_d.offset + BPC * OBS, [[NAUX, 1], [1, NAUX]]))
        k = 1
        while k < PT:
            nc.sync.dma_start(aux_t[k:2 * k, :], aux_t[0:k, :])
            k *= 2

        # sign rows for <Z_w>
        for w in range(6):
            r = 1 << (5 - w)
            nc.vector.memset(sg_t[:, w * 64:(w + 1) * 64], 1.0)
            neg = bass.AP(
                sg_t[:].tensor, sg_t[:].offset + w * 64 + r,
                [list(sg_t[:].ap)[0], [2 * r, 32 // r], [1, r]])
            nc.vector.memset(neg, -1.0)

        PLANE = {"A1": 0, "A2": 1, "B1": 2, "B2": 3, "NA2": 4, "NB1": 5}

        for tno in range(NTILES):
            x_t = io_p.tile([PT, OBS], f32, tag="x")
            out6_t = io_p.tile([PT, NA], f32, tag="out6")
            nc.sync.dma_start(x_t[:], xa_d[tno * PT:(tno + 1) * PT, :])

            # --- coefficient planes ---------------------------------------
            # alpha[l,w] = ISC1*x ; beta = ISC2*x + WT1 ; p/m = alpha +- VH
            X5 = W_t[:, 0:NW]      # temp: x tiled 5x
            for l in range(NL):
                nc.vector.tensor_copy(X5[:, l * NQ:(l + 1) * NQ], x_t[:])
            alpha = ANG_t[:, 0:NW]  # temp
            nc.vector.tensor_mul(alpha, X5, aux_t[:, 0:NW])
            beta = W_t[:, 2 * NW:3 * NW]
            nc.vector.tensor_mul(beta, X5, aux_t[:, NW:2 * NW])
            nc.vector.tensor_add(beta, beta, aux_t[:, 2 * NW:3 * NW])
            p_ = W_t[:, 0:NW]      # overwrites X5 (alpha already extracted)
            m_ = W_t[:, NW:2 * NW]
            nc.vector.tensor_add(p_, alpha, aux_t[:, 3 * NW:4 * NW])
            nc.vector.tensor_sub(m_, alpha, aux_t[:, 3 * NW:4 * NW])

            # trig over [p | m | beta] (210 cols): wrap + Horner
            y = ANG_t[:, 0:3 * NW]
            t2 = ANG_t[:, 3 * NW:6 * NW]
            aC = CS_t[:, 0:3 * NW]
            aS = CS_t[:, 3 * NW:6 * NW]
            nc.vector.add_range_wrap(y, W_t[:, 0:3 * NW], 0.0, PI, 2.0 * PI)
            nc.vector.tensor_mul(t2, y, y)
            nc.vector.tensor_scalar(aS, t2, SIN_P[0], SIN_P[1], mul_op, add_op)
            for ck in SIN_P[2:]:
                nc.vector.tensor_mul(aS, aS, t2)
                nc.vector.tensor_scalar_add(aS, aS, ck)
            nc.vector.tensor_mul(aS, aS, y)
            nc.vector.tensor_scalar(aC, t2, COS_Q[0], COS_Q[1], mul_op, add_op)
            for ck in COS_Q[2:]:
                nc.vector.tensor_mul(aC, aC, t2)
                nc.vector.tensor_scalar_add(aC, aC, ck)

            cosp, cosm, cosb = (aC[:, 0:NW], aC[:, NW:2 * NW], aC[:, 2 * NW:3 * NW])
            sinp, sinm, sinb = (aS[:, 0:NW], aS[:, NW:2 * NW], aS[:, 2 * NW:3 * NW])
            A1 = PL_t[:, 0:NW]
            A2 = PL_t[:, NW:2 * NW]
            B1 = PL_t[:, 2 * NW:3 * NW]
            B2 = PL_t[:, 3 * NW:4 * NW]
            NA2v = PL_t[:, 4 * NW:5 * NW]
            NB1v = PL_t[:, 5 * NW:6 * NW]
            nc.vector.tensor_mul(A1, cosb, cosp)
            nc.vector.tensor_mul(A2, sinb, cosm)
            nc.vector.tensor_mul(B1, cosb, sinp)
            nc.vector.tensor_mul(B2, sinb, sinm)
            nc.vector.tensor_scalar_mul(NA2v, A2, -1.0)
            nc.vector.tensor_scalar_mul(NB1v, B1, -1.0)

            # --- state init (also initializes the cmulign dummy-in1 read
            # regions) + gates ----------------------------------------------
            nc.vector.memset(A_t[:], 0.0)

            tiles = {"A": A_t[:], "B": B_t[:], "T": T_t[:], "S": s64_t[:]}

            def ap(reg):
                return _ap(bass, tiles[reg[0]], reg)

            def scal(ref):
                pli, col = PLANE[ref[0]], ref[1]
                return PL_t[:, pli * NW + col:pli * NW + col + 1]

            for op in sched:
                kind = op[0]
                if kind == "seed":
                    nc.vector.memset(T_t[:, 0:1], 1.0)
                    nc.vector.memset(T_t[:, 1:2], 0.0)
                elif kind == "cmulign":
                    _, dst, src, c0, c1 = op
                    _check_pair_eligible(dst)
                    _check_pair_eligible(src)
                    n = _nelem(src)
                    dummy = ("A", 0, ((1, n),))
                    emit_pair("cmulign", ap(dst), ap(src), ap(dummy),
                              scal(c0), scal(c1))
                elif kind == "cmulacc":
                    _, dst, src, acc, c0, c1 = op
                    _check_pair_eligible(dst)
                    _check_pair_eligible(src)
                    _check_pair_eligible(acc)
                    assert len(acc[2]) == 1
                    emit_pair("cmulacc", ap(dst), ap(src), ap(acc),
                              scal(c0), scal(c1))
                elif kind == "sqsum":
                    _, dst, s0, s1 = op
                    nc.vector._custom_dve(
                        cops["sqsum"], out=ap(dst), in0=ap(s0), in1=ap(s1))
                elif kind == "red":
                    _, dst, src = op
                    nc.vector.tensor_reduce(
                        ap(dst), ap(src), axis=mybir.AxisListType.X,
                        op=mybir.AluOpType.add)
                else:
                    raise ValueError(kind)

            for w in range(NA):
                nc.vector.tensor_mul(
                    r64_t[:], s64_t[:], sg_t[:, w * 64:(w + 1) * 64])
                nc.vector.tensor_reduce(
                    out6_t[:, w:w + 1], r64_t[:],
                    axis=mybir.AxisListType.X, op=mybir.AluOpType.add)
            nc.sync.dma_start(out_d[tno * PT:(tno + 1) * PT, :], out6_t[:])

    # stamp perf_max=1 post-Tile (scheduling rebuilds instructions)
    names = set(pm_names)
    n_pm = 0
    for fn in nc.m.functions:
        for blk in fn.blocks:
            for inst in blk.instructions:
                if type(inst).__name__ == "InstCustomDveAnt" and inst.name in names:
                    inst.perf_max = 1
                    n_pm += 1
    assert n_pm == len(names), (n_pm, len(names))
    nc.compile()
    return nc


# ------------------------------------------------------------------- runner

_NC_CACHE = None
_RUNNER = None


class _Result:
    exec_time_ns = None


class Runner:
    """Persistent jitted SPMD executor (cached shard_map closure)."""

    def __init__(self, nc, n_cores=NCORES):
        import jax
        from jax.sharding import Mesh, PartitionSpec
        from jax.experimental.shard_map import shard_map
        from concourse import bass2jax
        import concourse.mybir as mybir

        bass2jax.install_neuronx_cc_hook()
        self.nc = nc
        self.n_cores = n_cores
        part_name = nc.partition_id_tensor.name if nc.partition_id_tensor else None
        in_names, out_names, out_avals, self.zero_shapes = [], [], [], []
        for alloc in nc.m.functions[0].allocations:
            if not isinstance(alloc, mybir.MemoryLocationSet):
                continue
            name = alloc.memorylocations[0].name
            if alloc.kind == "ExternalInput":
                if name != part_name:
                    in_names.append(name)
            elif alloc.kind == "ExternalOutput":
                out_names.append(name)
                shape = tuple(alloc.tensor_shape)
                dtype = mybir.dt.np(alloc.dtype)
                out_avals.append(jax.core.ShapedArray(shape, dtype))
                self.zero_shapes.append((shape, dtype))
        self.in_names = list(in_names)
        self.out_names = list(out_names)
        n_params = len(in_names)
        n_outs = len(out_names)
        all_in = list(in_names) + list(out_names)
        if part_name is not None:
            all_in.append(part_name)
        donate = tuple(range(n_params, n_params + n_outs))

        def _body(*args):
            operands = list(args)
            if nc.partition_id_tensor is not None:
                operands.append(bass2jax.partition_id_tensor())
            outs = bass2jax._bass_exec_p.bind(
                *operands,
                out_avals=tuple(out_avals),
                in_names=tuple(all_in),
                out_names=tuple(out_names),
                lowering_input_output_aliases=(),
                sim_require_finite=True,
                sim_require_nnan=True,
                nc=nc,
            )
            return tuple(outs)

        devices = jax.devices()[:n_cores]
        mesh = Mesh(np.asarray(devices), ("core",))
        in_specs = (PartitionSpec("core"),) * (n_params + n_outs)
        out_specs = (PartitionSpec("core"),) * n_outs
        self.fn = jax.jit(
            shard_map(_body, mesh=mesh, in_specs=in_specs,
                      out_specs=out_specs, check_rep=False),
            donate_argnums=donate,
            keep_unused=True,
        )

    def _sharding(self):
        import jax
        from jax.sharding import Mesh, NamedSharding, PartitionSpec
        if getattr(self, "_shard", None) is None:
            mesh = Mesh(np.asarray(jax.devices()[:self.n_cores]), ("core",))
            self._shard = NamedSharding(mesh, PartitionSpec("core"))
        return self._shard

    def _stage_zeros(self):
        # donated output buffers, staged on device between calls so the
        # timed call does no host->device zero transfer
        import jax
        sh = self._sharding()
        self._zeros = [
            jax.device_put(np.zeros((self.n_cores * s[0], *s[1:]), d), sh)
            for s, d in self.zero_shapes
        ]

    def put_input(self, arr):
        import jax
        return jax.device_put(arr, self._sharding())

    def __call__(self, concat_in):
        if getattr(self, "_zeros", None) is None:
            self._stage_zeros()
        zeros = self._zeros
        self._zeros = None
        out_arrs = self.fn(*concat_in, *zeros)
        res = [np.asarray(o) for o in out_arrs]
        self._stage_zeros()  # async restage for the next call
        return res


def run_cores(x_full, aux):
    global _NC_CACHE, _RUNNER
    if _NC_CACHE is None:
        _NC_CACHE = build_bass()
    nc = _NC_CACHE
    xs = np.asarray(x_full, np.float32).reshape(NCORES, BPC, OBS)
    aux_rows = np.asarray(aux, np.float32).reshape(1, NAUX // OBS, OBS)
    xa = np.concatenate(
        [xs, np.broadcast_to(aux_rows, (NCORES, NAUX // OBS, OBS))], axis=1)
    xa = np.ascontiguousarray(xa.reshape(NCORES * (BPC + NAUX // OBS), OBS))
    last_err = None
    for attempt in range(3):
        try:
            if _RUNNER is None:
                _RUNNER = Runner(nc)
            # fresh async device_put each call: overlaps the transfer with
            # dispatch. (Measured: REUSING a committed device array across
            # calls is ~25 ms slower through axon than re-putting fresh.)
            results = _RUNNER([_RUNNER.put_input(xa)])
            break
        except Exception as e:
            last_err = e
            if _RUNNER is not None:
                _RUNNER._zeros = None
            import time as _time
            _time.sleep(45 * (attempt + 1))
    else:
        raise last_err
    return results[0], _Result()


def kernel(x, input_scaling, weights, action_scale, action_bias):
    aux = make_aux(input_scaling, weights)
    s6, _ = run_cores(np.asarray(x, np.float32), aux)
    return s6 * np.asarray(action_scale, np.float32) + np.asarray(
        action_bias, np.float32)


# revision 14
# speedup vs baseline: 1.1214x; 1.1214x over previous
"""Trainium2 Bass kernel v2: 14-qubit data-reuploading quantum circuit actor.

Core idea vs v1: hand-authored 2x_1p custom-DVE *pair* ops on interleaved
complex fp16 — lo/hi lanes of the packed-fp16 datapath compute (re, im) of a
complex multiply, so a full merged per-wire gate U = RY(v)RZ(b)RY(a) is 4 fat
instructions at 2 elems/cycle:

    T   = U00 (x) X        (CMULIGN:  out = (C0+iC1) (x) in0)
    B.X = U01 (x) Y + T    (CMULACC:  out = (C0+iC1) (x) in0 + in1)
    T   = U10 (x) X
    B.Y = U11 (x) Y + T

U structure (alpha = RY input half-angle, beta = RZ half-angle incl weight-RZ,
vh = weight-RY half-angle): with p = alpha+vh, m = alpha-vh:
    A1 = cos(beta) cos(p); A2 = sin(beta) cos(m)
    B1 = cos(beta) sin(p); B2 = sin(beta) sin(m)
    U00 = A1 - i A2 ; U01 = -B1 + i B2 ; U10 = B1 + i B2 ; U11 = A1 + i A2

CNOT(t-1, t) of the ring folds into wire-t's write APs (region split on bit
t-1); CNOT(13, 0) folds into the next layer's wire-0 reads (and into the
measurement reads for the last layer). perf_max=1 is stamped on each pair-op
instruction post-Tile so the RTL engages the 2x_1p uop slot (validated on HW:
the 2x program's pair semantics only appear with perf_max=1).

Layer 1 acts on |0..0>, so it is replaced by a product-state doubling build
(~54 tiny ops instead of 14 full-state gates), with the ring folded into the
append APs.

Inputs shipped per call: ONE packed f32 array per core (256 rows of x +
20 rows holding the 280-float aux table of host-precomputed isc/2, weights/2
terms), ~124KB total vs 1.7MB for the v1 angle table. Angles + trig
(range-wrap + Horner minimax) + the 6 coefficient planes are computed
on-chip per 128-row tile.

Per-wire phase normalization: each gate U is multiplied by the unit phase
k = conj(U10)/|U10| (global phase is unobservable in the probabilities),
making U10' real — so the second T-pass runs as a stock tensor_scalar at
4 elem/cycle instead of a 2x pair op. |U10| is computed on-chip via ACT
sqrt + one Newton step (+1e-12 guard on B1 for the U10=0 corner).

Measured: CoreSim exec 3.64 ms/core (v1 baseline: 19.0 ms). HW warm
wall 54-70 ms/call (load-dependent), ~50-60 ms of which is the fixed
axon-tunnel dispatch+fetch round trip (invariant to core count, bytes, or
work). Call path: fresh async device_put of the packed input each call
(reusing a committed device array is ~25 ms slower through axon) + donated
zero output buffers re-staged on device between calls.
Relative rms error vs reference: 3.0e-3 (v1: 4.0e-3; gate: 2e-2).
"""

import numpy as np

NQ = 14
NL = 5
OBS = 14
NA = 6
B = 2048
NCORES = 8
BPC = B // NCORES          # 256 rows per core
PT = 128                   # partitions per tile
NTILES = BPC // PT         # 2
NS = 1 << NQ               # 16384 amplitudes
F = 2 * NS                 # 32768 floats per row (interleaved complex)
NW = NL * NQ               # 70 (layer, wire) pairs
PI = float(np.pi)

# aux layout (floats, [1, 4*NW]): ISC1 | ISC2 | WT1 | VH
#   ISC1[l,w] = input_scaling[l,w]/2
#   ISC2[l,w] = input_scaling[l,w+14]/2
#   WT1[l,w]  = weights[l,w]/2
#   VH[l,w]   = weights[l,w+14]/2
NAUX = 4 * NW

SIN_P = [2.2248706406891887e-06, -0.00019424154210166545,
         0.008319842398281522, -0.16665145941120196,
         0.9999972898367918]
COS_Q = [-2.219394993734796e-07, 2.42531924958235e-05,
         -0.001386274731586208, 0.04166103279007339,
         -0.4999955816555398, 0.9999994436793969]


def make_aux(input_scaling, weights):
    isc = np.asarray(input_scaling, np.float64)
    wt = np.asarray(weights, np.float64)
    aux = np.concatenate([
        (isc[:, :NQ] / 2.0).ravel(),
        (isc[:, NQ:] / 2.0).ravel(),
        (wt[:, :NQ] / 2.0).ravel(),
        (wt[:, NQ:] / 2.0).ravel(),
    ]).astype(np.float32)
    return aux.reshape(1, NAUX)


def coef_planes(x, aux):
    """Host/numpy mirror of the on-chip coef computation (float64 path).
    x: (n, 14) -> dict of (n, 70) planes A1, A2, B1, B2."""
    x = np.asarray(x, np.float64)
    a = np.asarray(aux, np.float64).ravel()
    isc1 = a[0:NW].reshape(NL, NQ)
    isc2 = a[NW:2 * NW].reshape(NL, NQ)
    wt1 = a[2 * NW:3 * NW].reshape(NL, NQ)
    vh = a[3 * NW:4 * NW].reshape(NL, NQ)
    xb = x[:, None, :]
    alpha = isc1[None] * xb
    beta = isc2[None] * xb + wt1[None]
    p = alpha + vh[None]
    m = alpha - vh[None]
    cb, sb = np.cos(beta), np.sin(beta)
    A1 = cb * np.cos(p)
    A2 = sb * np.cos(m)
    B1 = cb * np.sin(p)
    B2 = sb * np.sin(m)
    # per-wire global phase k = conj(U10g)/|U10g| (U10g = B1g + iB2) makes
    # U10' real (= S); global phase is unobservable in the probabilities.
    B1g = B1 + 1e-12
    S = np.sqrt(B1g * B1g + B2 * B2)
    rinv = 1.0 / S
    kr = B1g * rinv
    q = B2 * rinv          # ki = -q
    out = {
        "U00r": kr * A1 - q * A2,
        "U00i": -(q * A1 + kr * A2),
        "U01r": -(kr * B1 - q * B2),
        "U01i": kr * B2 + q * B1,
        "U11r": kr * A1 + q * A2,
        "U11i": kr * A2 - q * A1,
        "S": S,
    }
    return {k: v.reshape(-1, NW).astype(np.float32) for k, v in out.items()}


# ---------------------------------------------------------------- schedule
# region = (buf, float_offset, dims); dims = ((step, count), ...) innermost
# last, float-index space. buf: "A"/"B" state, "T" scratch (16384 floats),
# "S" s64 sums. Every pair-op region: innermost step 1, even count, even
# offset (2x_1p eligibility).
#
# ops:
#  ("cmulign", dst, src, c0, c1): dst = (c0+ic1) (x) src      [in1 ignored]
#  ("cmulacc", dst, src, acc, c0, c1): dst = (c0+ic1)(x)src + acc (acc rank-1)
#  ("sqsum", dst, s0, s1): dst = s0^2 + s1^2
#  ("red", dst, src): 64-block reduce
# scalar ref = (plane, col), plane in A1,A2,B1,B2,NA2,NB1.


def _norm(dims):
    d = [(s, c) for s, c in dims if c != 1]
    out = []
    for s, c in d:
        if out and out[-1][0] == s * c:
            out[-1] = (s, c * out[-1][1])
        else:
            out.append((s, c))
    if not out:
        out = [(1, 1)]
    assert len(out) <= 2, out
    return tuple(out)


def _reg(buf, off, *dims):
    return (buf, off, _norm(dims))


def _nelem(reg):
    n = 1
    for _, c in reg[2]:
        n *= c
    return n


class Sched:
    def __init__(self):
        self.ops = []
        self.cur = "A"

    def swap(self):
        self.cur = "B" if self.cur == "A" else "A"

    def gate(self, l, t):
        """Merged U(l, t) with ring-fold on writes (t>=1) and prev-layer
        C(13,0) fold on reads (t==0, l>=1)."""
        a, b = self.cur, "B" if self.cur == "A" else "A"
        col = l * NQ + t
        U00 = (("U00r", col), ("U00i", col))
        U01 = (("U01r", col), ("U01i", col))
        U11 = (("U11r", col), ("U11i", col))
        Ssc = ("S", col)

        if t == 0:
            if l == 0:
                X = [_reg(a, 0, (1, NS))]
                Y = [_reg(a, NS, (1, NS))]
                DX = [_reg(b, 0, (1, NS))]
                DY = [_reg(b, NS, (1, NS))]
            else:
                d = ((4, NS // 4), (1, 2))
                X = [_reg(a, 0, *d), _reg(a, NS + 2, *d)]
                Y = [_reg(a, NS, *d), _reg(a, 2, *d)]
                DX = [_reg(b, 0, *d), _reg(b, 2, *d)]
                DY = [_reg(b, NS, *d), _reg(b, NS + 2, *d)]
        else:
            Ft = 1 << (14 - t)
            nb = 1 << (t - 1)
            d = ((4 * Ft, nb), (1, Ft))
            X = [_reg(a, 0, *d), _reg(a, 2 * Ft, *d)]
            Y = [_reg(a, Ft, *d), _reg(a, 3 * Ft, *d)]
            # ring C(t-1, t): odd-b (bit t-1 = 1) writes land bit-t-flipped
            DX = [_reg(b, 0, *d), _reg(b, 3 * Ft, *d)]    # out0 -> X | Yo
            DY = [_reg(b, Ft, *d), _reg(b, 2 * Ft, *d)]   # out1 -> Y | Xo

        nparts = len(X)
        half = NS if nparts == 1 else NS // 2
        for i in range(nparts):
            Ti = _reg("T", i * half, (1, half))
            self.ops.append(("cmulign", Ti, X[i], U00[0], U00[1]))   # U00' (x) X
            self.ops.append(("cmulacc", DX[i], Y[i], Ti, U01[0], U01[1]))
        for i in range(nparts):
            Ti = _reg("T", i * half, (1, half))
            self.ops.append(("tsmul", Ti, X[i], Ssc))                # S * X (real)
            self.ops.append(("cmulacc", DY[i], Y[i], Ti, U11[0], U11[1]))
        self.swap()

    def layer0_build(self):
        """Layer-1 on |0..0>: product state via doubling, appending qubit w
        as the new innermost index; ring C(w-1, w) folds into the append APs
        (odd source index j <-> bit w-1 = 1 -> flip new bit w).
        s_0..s_11 ping-pong in T halves, s_12 -> B[0:16384], s_13 -> A."""
        assert self.cur == "A"
        ops = self.ops
        # seed: T[0:2] = (1, 0) — emitted by the bass builder (memset), and
        # by the numpy executor, via the special op below.
        ops.append(("seed",))

        def v0(w):
            return ("U00r", w), ("U00i", w)   # U00' column entry

        def v1(w):
            return ("S", w), None             # U10' = S (real)

        def place(k):
            # buffer holding s_k (size 2**(k+2) floats)
            if k <= 11:
                return ("T", 8192 * (k % 2))
            if k == 12:
                return ("B", 0)
            return ("A", 0)

        # qubit 0: s_0 from seed (no fold)
        c0, s0 = v0(0)
        c1, s1 = v1(0)
        seed = _reg("T", 0, (1, 2))
        dstb, dsto = place(0)
        # b=1 first (disjoint), then b=0 in-place over the seed
        ops.append(("cmulign", _reg(dstb, dsto + 2, (4, 1), (1, 2)), seed, c1, s1))
        ops.append(("cmulign", _reg(dstb, dsto + 0, (4, 1), (1, 2)), seed, c0, s0))
        for w in range(1, NQ):
            sb, so = place(w - 1)
            db, do = place(w)
            nE = 1 << (w - 1)  # even-j count == odd-j count
            srcE = _reg(sb, so + 0, (4, nE), (1, 2))
            srcO = _reg(sb, so + 2, (4, nE), (1, 2))
            c0, s0 = v0(w)
            c1, s1 = v1(w)
            # b=0 (U00 factor): even j -> 2j ; odd j -> 2j+1 (bit-w flip)
            ops.append(("cmulign", _reg(db, do + 0, (8, nE), (1, 2)), srcE, c0, s0))
            ops.append(("cmulign", _reg(db, do + 6, (8, nE), (1, 2)), srcO, c0, s0))
            # b=1 (U10 factor): even j -> 2j+1 ; odd j -> 2j
            ops.append(("cmulign", _reg(db, do + 2, (8, nE), (1, 2)), srcE, c1, s1))
            ops.append(("cmulign", _reg(db, do + 4, (8, nE), (1, 2)), srcO, c1, s1))
        # s_13 landed in A; cur stays "A"

    def measurement(self):
        a = self.cur
        self.ops.append(("sqsum", _reg("T", 0, (2, 8192)),
                         _reg(a, 0, (4, 8192)), _reg(a, 1, (4, 8192))))
        self.ops.append(("sqsum", _reg("T", 1, (2, 4096)),
                         _reg(a, NS + 2, (4, 4096)), _reg(a, NS + 3, (4, 4096))))
        self.ops.append(("sqsum", _reg("T", 8193, (2, 4096)),
                         _reg(a, 2, (4, 4096)), _reg(a, 3, (4, 4096))))
        self.ops.append(("red", ("S", 0, ((1, 64),)),
                         ("T", 0, ((256, 64), (1, 256)))))


def build_schedule():
    S = Sched()
    S.layer0_build()
    for l in range(1, NL):
        for t in range(NQ):
            S.gate(l, t)
    S.measurement()
    return S.ops


# ------------------------------------------------------------ numpy executor


def _indices(reg):
    _, off, dims = reg
    idx = np.array([0], np.int64)
    for st, ct in dims:
        idx = (idx[:, None] + (np.arange(ct, dtype=np.int64) * st)[None, :]).ravel()
    return off + idx


def simulate_numpy(x, aux, fp16=True):
    """x: (n, 14) -> (n, 64) block sums, mirroring the on-device schedule."""
    pl = coef_planes(x, aux)
    n = x.shape[0]
    sdt = np.float16 if fp16 else np.float32
    bufs = {
        "A": np.zeros((n, F), sdt),
        "B": np.zeros((n, F), sdt),
        "T": np.zeros((n, NS), sdt),
        "S": np.zeros((n, 64), np.float32),
    }
    bufs["A"][:, 0] = 1.0

    def cmul(src_v, c, s):
        lo, hi = src_v[:, 0::2], src_v[:, 1::2]
        out = np.empty_like(src_v)
        out[:, 0::2] = c * lo - s * hi
        out[:, 1::2] = s * lo + c * hi
        return out

    for op in build_schedule():
        kind = op[0]
        if kind == "seed":
            bufs["T"][:, 0] = 1.0
            bufs["T"][:, 1] = 0.0
        elif kind in ("cmulign", "cmulacc"):
            if kind == "cmulign":
                _, dst, src, c0, c1 = op
                acc_v = 0.0
            else:
                _, dst, src, acc, c0, c1 = op
                acc_v = bufs[acc[0]][:, _indices(acc)].astype(np.float32)
            c = pl[c0[0]][:n, c0[1]:c0[1] + 1].astype(np.float32)
            if c1 is None:
                s = np.zeros_like(c)
            else:
                s = pl[c1[0]][:n, c1[1]:c1[1] + 1].astype(np.float32)
            src_v = bufs[src[0]][:, _indices(src)].astype(np.float32)
            v = cmul(src_v, c, s) + acc_v
            bufs[dst[0]][:, _indices(dst)] = v.astype(sdt)
        elif kind == "tsmul":
            _, dst, srcr, sc = op
            c = pl[sc[0]][:n, sc[1]:sc[1] + 1].astype(np.float32)
            v = c * bufs[srcr[0]][:, _indices(srcr)].astype(np.float32)
            bufs[dst[0]][:, _indices(dst)] = v.astype(sdt)
        elif kind == "sqsum":
            _, dst, s0, s1 = op
            v = (bufs[s0[0]][:, _indices(s0)].astype(np.float32) ** 2
                 + bufs[s1[0]][:, _indices(s1)].astype(np.float32) ** 2)
            bufs[dst[0]][:, _indices(dst)] = v.astype(sdt)
        elif kind == "red":
            _, dst, src = op
            v = bufs[src[0]][:, _indices(src)].astype(np.float32)
            bufs["S"][:, _indices(dst)] = v.reshape(n, 64, 256).sum(axis=2)
        else:
            raise ValueError(kind)
    return bufs["S"].copy()


def postprocess(s64, action_scale, action_bias):
    blk = np.arange(64)
    out = np.zeros((s64.shape[0], NA), np.float32)
    for w in range(NA):
        sign = 1.0 - 2.0 * ((blk >> (5 - w)) & 1)
        out[:, w] = s64 @ sign.astype(np.float32)
    return out * np.asarray(action_scale, np.float32) + np.asarray(
        action_bias, np.float32)


# ------------------------------------------------------------------ DVE ops

_CUSTOM = {}


def _build_pair_uop(with_acc):
    from concourse.dve_uop import (
        InpSel, OutSel, AluInp as D, DelayInp, OutPath, Trigger, UopConfig,
        UopDpConfig, AluOp, ENABLE)

    def dp(op, a, b, capture=None, passes=()):
        d = UopDpConfig().enable_alu(op, a, b)
        if capture is not None:
            d.enable_delay_from_src(DelayInp.PREV_ALU_OUT, capture)
        if passes:
            d.pass_through_delay(*passes)
        return d

    u = UopConfig()
    u.enable_input(InpSel.SRC_0, 1)      # d0 = X_lo
    u.enable_input(InpSel.CONST_0, 2)    # d1 = C0
    u.enable_input(InpSel.SRC_0_HI, 3)   # d2 = X_hi
    u.enable_input(InpSel.CONST_1, 4)    # d3 = C1
    u.enable_input(InpSel.SRC_1, 5)      # d4 = T_lo
    u.enable_input(InpSel.SRC_1_HI, 6)   # d5 = T_hi
    if with_acc:
        u.datapath_config[0] = dp(AluOp.MULTIPLY, D.PREV_DELAY_0, D.PREV_DELAY_1,
                                  passes=(0, 1, 2, 3, 4, 5))
        u.datapath_config[1] = dp(AluOp.ADD, D.PREV_ALU_OUT, D.PREV_DELAY_4,
                                  passes=(0, 1, 2, 3, 5))
        u.datapath_config[2] = dp(AluOp.MULTIPLY, D.PREV_DELAY_2, D.PREV_DELAY_3,
                                  capture=4, passes=(0, 1, 2, 3, 5))
        u.datapath_config[3] = dp(AluOp.SUBTRACT, D.PREV_DELAY_4, D.PREV_ALU_OUT,
                                  passes=(0, 1, 2, 3, 5))
        u.datapath_config[4] = dp(AluOp.MULTIPLY, D.PREV_DELAY_0, D.PREV_DELAY_3,
                                  capture=4, passes=(1, 2, 5))
        u.datapath_config[5] = dp(AluOp.MULTIPLY, D.PREV_DELAY_2, D.PREV_DELAY_1,
                                  capture=0, passes=(4, 5))
        u.datapath_config[6] = dp(AluOp.ADD, D.PREV_DELAY_0, D.PREV_ALU_OUT,
                                  passes=(4, 5))
        u.datapath_config[7] = dp(AluOp.ADD, D.PREV_ALU_OUT, D.PREV_DELAY_5,
                                  passes=(4,))
    else:
        u.datapath_config[0] = dp(AluOp.MULTIPLY, D.PREV_DELAY_0, D.PREV_DELAY_1,
                                  passes=(0, 1, 2, 3))
        u.datapath_config[1] = dp(AluOp.MULTIPLY, D.PREV_DELAY_2, D.PREV_DELAY_3,
                                  capture=4, passes=(0, 1, 2, 3))
        u.datapath_config[2] = dp(AluOp.SUBTRACT, D.PREV_DELAY_4, D.PREV_ALU_OUT,
                                  passes=(0, 1, 2, 3))
        u.datapath_config[3] = dp(AluOp.MULTIPLY, D.PREV_DELAY_0, D.PREV_DELAY_3,
                                  capture=4, passes=(1, 2))
        u.datapath_config[4] = dp(AluOp.MULTIPLY, D.PREV_DELAY_2, D.PREV_DELAY_1,
                                  capture=5, passes=(4,))
        u.datapath_config[5] = dp(AluOp.ADD, D.PREV_DELAY_5, D.PREV_ALU_OUT,
                                  passes=(4,))
        u.datapath_config[6] = dp(AluOp.BYPASS, D.PREV_ALU_OUT, D.PREV_ALU_OUT,
                                  passes=(4,))
        u.datapath_config[7] = dp(AluOp.BYPASS, D.PREV_ALU_OUT, D.PREV_ALU_OUT,
                                  passes=(4,))
    u.enable_output(OutSel.DELAY_4, OutPath.WR0_LO)
    u.enable_output(OutSel.ALU_OUT, OutPath.WR0_HI)
    u.require_inp0 = ENABLE
    u.require_inp1 = ENABLE
    u.trigger = (Trigger.SRC_TENSOR_DONE, Trigger.NONE, Trigger.NONE)
    u.next_uop = (0, 0, 0)
    return u


def _sc_np(s, p):
    s = np.asarray(s, np.float32)
    return s.reshape(p, -1) if s.size > 1 else s.reshape(-1)


def _cmulacc_ref(in0, in1, s0, s1, imm2):
    p = in0.shape[0]
    x = np.asarray(in0, np.float32).reshape(p, -1)
    t = np.asarray(in1, np.float32).reshape(p, -1)
    c, s = _sc_np(s0, p), _sc_np(s1, p)
    out = np.empty_like(x)
    out[:, 0::2] = c * x[:, 0::2] - s * x[:, 1::2] + t[:, 0::2]
    out[:, 1::2] = s * x[:, 0::2] + c * x[:, 1::2] + t[:, 1::2]
    return out.reshape(in0.shape)


def _cmulign_ref(in0, in1, s0, s1, imm2):
    p = in0.shape[0]
    x = np.asarray(in0, np.float32).reshape(p, -1)
    c, s = _sc_np(s0, p), _sc_np(s1, p)
    out = np.empty_like(x)
    out[:, 0::2] = c * x[:, 0::2] - s * x[:, 1::2]
    out[:, 1::2] = s * x[:, 0::2] + c * x[:, 1::2]
    return out.reshape(in0.shape)


def _sqsum_ref(in0, in1, s0, s1, imm2):
    p = in0.shape[0]
    a = np.asarray(in0, np.float32).reshape(p, -1)
    b = np.asarray(in1, np.float32).reshape(p, -1)
    return (a * a + b * b).reshape(in0.shape)


def _get_custom_ops():
    if _CUSTOM:
        return _CUSTOM
    from concourse import dve_ops
    from concourse.dve_ops import DveOp, OPS
    from concourse.dve_spec import Spec, Src0, Src1, C0, C1, sq, lower
    from concourse.dve_uop import DveOpSpec

    _SPEC_CACHE = {}

    def register(name, body, ref, uop2x):
        for op in OPS:
            if op.name == name:
                return op
        row = dve_ops._CUSTOM_DVE_ROW_BASE + len(OPS)
        spec = Spec(body=body, reference=ref)

        if uop2x is not None:
            class DveOpPair(DveOp):
                def compile(self, ver):
                    key = (self.name, ver)
                    if key in _SPEC_CACHE:
                        return _SPEC_CACHE[key]
                    s = DveOpSpec(
                        name=self.name,
                        opcode=dve_ops.get_dve_sub_opcode(self.name),
                        uops=lower(self.spec, ver=ver),
                        uops_2x=[uop2x],
                        perf_max=1,
                        rd1_en=True,
                    )
                    got = s.sha(ver)
                    if self.uops_sha.get(ver) != got:
                        raise ValueError(f"{self.name}: sha drift {got}")
                    _SPEC_CACHE[key] = s
                    return s
            cls = DveOpPair
        else:
            cls = DveOp
        shas = {}
        for ver in ("v3", "v4"):
            kw = dict(uops_2x=[uop2x], perf_max=1) if uop2x is not None else {}
            s = DveOpSpec(name=name, opcode=row, uops=lower(spec, ver=ver),
                          rd1_en=True, **kw)
            shas[ver] = s.sha(ver)
        op = cls(name, spec, subdim=False, uops_sha=shas)
        OPS.append(op)
        dve_ops._SUB_OPCODE_FOR_NAME[name] = row
        dve_ops.CUSTOM_DVE_SPECS[name] = spec
        return op

    # 1x placeholder bodies are flat (wrong for pair semantics) — correctness
    # depends on the 2x slot engaging; emitter asserts AP eligibility.
    _CUSTOM["cmulacc"] = register(
        "CMULACC_K", Src0 * C0 + Src1 * C1, _cmulacc_ref, _build_pair_uop(True))
    _CUSTOM["cmulign"] = register(
        "CMULIGN_K", Src0 * C0 + Src1 * C1, _cmulign_ref, _build_pair_uop(False))
    _CUSTOM["sqsum"] = register(
        "SQSUM_K", sq(Src0) + sq(Src1), _sqsum_ref, None)
    return _CUSTOM


# ------------------------------------------------------------------ bass side


def _ap(bass_mod, tile_ap, reg):
    t = tile_ap.tensor
    part = list(tile_ap.ap)[0]
    dims = [[part[0], part[1]]] + [[s, c] for s, c in reg[2]]
    return bass_mod.AP(t, tile_ap.offset + reg[1], dims)


def _check_pair_eligible(reg):
    _, off, dims = reg
    assert off % 2 == 0, reg
    st, ct = dims[-1]
    assert st == 1 and ct >= 2 and ct % 2 == 0, reg
    if len(dims) == 2:
        assert dims[0][0] % 2 == 0, reg


def build_bass():
    import concourse.bass as bass
    import concourse.mybir as mybir
    import concourse.tile as tile
    from concourse import bacc
    from contextlib import ExitStack

    f32 = mybir.dt.float32
    f16 = mybir.dt.float16
    cops = _get_custom_ops()
    sched = build_schedule()
    mul_op, add_op = mybir.AluOpType.mult, mybir.AluOpType.add

    nc = bacc.Bacc("TRN2", target_bir_lowering=False, debug=False)
    # packed input: rows 0..BPC-1 = x, rows BPC..BPC+19 = aux (NAUX=280
    # floats as 20 rows of 14); one array per call = one transfer
    xa_d = nc.dram_tensor("xa", [BPC + NAUX // OBS, OBS], f32,
                          kind="ExternalInput").ap()
    out_d = nc.dram_tensor("out", [BPC, NA], f32, kind="ExternalOutput").ap()

    pm_names = []

    def emit_pair(kind, dst_ap, src_ap, in1_ap, s0, s1):
        inst = nc.vector._custom_dve(
            cops[kind], out=dst_ap, in0=src_ap, in1=in1_ap, s0=s0, s1=s1)
        raw = inst.ins if hasattr(inst, "ins") else inst
        pm_names.append(raw.name)
        return inst

    with tile.TileContext(nc) as tc, ExitStack() as ctx:
        state_p = ctx.enter_context(tc.tile_pool(name="state", bufs=1))
        io_p = ctx.enter_context(tc.tile_pool(name="io", bufs=2))

        A_t = state_p.tile([PT, F], f16, tag="A")
        B_t = state_p.tile([PT, F], f16, tag="B")
        T_t = state_p.tile([PT, NS], f16, tag="T")
        aux_t = state_p.tile([PT, NAUX], f32, tag="aux")
        ANG_t = state_p.tile([PT, 6 * NW], f32, tag="ang")  # y(210) | t2(210)
        CS_t = state_p.tile([PT, 6 * NW], f32, tag="cs")   # cos(210) | sin(210)
        PL_t = state_p.tile([PT, 4 * NW], f32, tag="pl")   # A1 A2 B1 B2
        PL2_t = state_p.tile([PT, 7 * NW], f32, tag="pl2")  # U00 U01 U11 (r,i) | S
        W_t = state_p.tile([PT, 3 * NW], f32, tag="w")     # raw angles p|m|beta
        sg_t = state_p.tile([PT, 6 * 64], f32, tag="sg")
        s64_t = state_p.tile([PT, 64], f32, tag="s64")
        r64_t = state_p.tile([PT, 64], f32, tag="r64")

        # aux broadcast to all partitions: 1 DMA + 7 doubling DMAs
        nc.sync.dma_start(
            aux_t[0:1, :],
            bass.AP(xa_d.tensor, xa_d.offset + BPC * OBS, [[NAUX, 1], [1, NAUX]]))
        k = 1
        while k < PT:
            nc.sync.dma_start(aux_t[k:2 * k, :], aux_t[0:k, :])
            k *= 2

        # sign rows for <Z_w>
        for w in range(6):
            r = 1 << (5 - w)
            nc.vector.memset(sg_t[:, w * 64:(w + 1) * 64], 1.0)
            neg = bass.AP(
                sg_t[:].tensor, sg_t[:].offset + w * 64 + r,
                [list(sg_t[:].ap)[0], [2 * r, 32 // r], [1, r]])
            nc.vector.memset(neg, -1.0)

        PLANE = {"U00r": 0, "U00i": 1, "U01r": 2, "U01i": 3,
                 "U11r": 4, "U11i": 5, "S": 6}

        for tno in range(NTILES):
            x_t = io_p.tile([PT, OBS], f32, tag="x")
            out6_t = io_p.tile([PT, NA], f32, tag="out6")
            nc.sync.dma_start(x_t[:], xa_d[tno * PT:(tno + 1) * PT, :])

            # --- coefficient planes ---------------------------------------
            # alpha[l,w] = ISC1*x ; beta = ISC2*x + WT1 ; p/m = alpha +- VH
            X5 = W_t[:, 0:NW]      # temp: x tiled 5x
            for l in range(NL):
                nc.vector.tensor_copy(X5[:, l * NQ:(l + 1) * NQ], x_t[:])
            alpha = ANG_t[:, 0:NW]  # temp
            nc.vector.tensor_mul(alpha, X5, aux_t[:, 0:NW])
            beta = W_t[:, 2 * NW:3 * NW]
            nc.vector.tensor_mul(beta, X5, aux_t[:, NW:2 * NW])
            nc.vector.tensor_add(beta, beta, aux_t[:, 2 * NW:3 * NW])
            p_ = W_t[:, 0:NW]      # overwrites X5 (alpha already extracted)
            m_ = W_t[:, NW:2 * NW]
            nc.vector.tensor_add(p_, alpha, aux_t[:, 3 * NW:4 * NW])
            nc.vector.tensor_sub(m_, alpha, aux_t[:, 3 * NW:4 * NW])

            # trig over [p | m | beta] (210 cols): wrap + Horner
            y = ANG_t[:, 0:3 * NW]
            t2 = ANG_t[:, 3 * NW:6 * NW]
            aC = CS_t[:, 0:3 * NW]
            aS = CS_t[:, 3 * NW:6 * NW]
            nc.vector.add_range_wrap(y, W_t[:, 0:3 * NW], 0.0, PI, 2.0 * PI)
            nc.vector.tensor_mul(t2, y, y)
            nc.vector.tensor_scalar(aS, t2, SIN_P[0], SIN_P[1], mul_op, add_op)
            for ck in SIN_P[2:]:
                nc.vector.tensor_mul(aS, aS, t2)
                nc.vector.tensor_scalar_add(aS, aS, ck)
            nc.vector.tensor_mul(aS, aS, y)
            nc.vector.tensor_scalar(aC, t2, COS_Q[0], COS_Q[1], mul_op, add_op)
            for ck in COS_Q[2:]:
                nc.vector.tensor_mul(aC, aC, t2)
                nc.vector.tensor_scalar_add(aC, aC, ck)

            cosp, cosm, cosb = (aC[:, 0:NW], aC[:, NW:2 * NW], aC[:, 2 * NW:3 * NW])
            sinp, sinm, sinb = (aS[:, 0:NW], aS[:, NW:2 * NW], aS[:, 2 * NW:3 * NW])
            A1 = PL_t[:, 0:NW]
            A2 = PL_t[:, NW:2 * NW]
            B1 = PL_t[:, 2 * NW:3 * NW]
            B2 = PL_t[:, 3 * NW:4 * NW]
            nc.vector.tensor_mul(A1, cosb, cosp)
            nc.vector.tensor_mul(A2, sinb, cosm)
            nc.vector.tensor_mul(B1, cosb, sinp)
            nc.vector.tensor_mul(B2, sinb, sinm)
            # phase-normalize: k = conj(B1g + iB2)/S, S = |B1g + iB2|;
            # makes U10' = S real so op3 runs as stock tensor_scalar @4x.
            U00r = PL2_t[:, 0:NW]
            U00i = PL2_t[:, NW:2 * NW]
            U01r = PL2_t[:, 2 * NW:3 * NW]
            U01i = PL2_t[:, 3 * NW:4 * NW]
            U11r = PL2_t[:, 4 * NW:5 * NW]
            U11i = PL2_t[:, 5 * NW:6 * NW]
            Spl = PL2_t[:, 6 * NW:7 * NW]
            Bg = W_t[:, 0:NW]
            R2 = W_t[:, NW:2 * NW]
            t1 = W_t[:, 2 * NW:3 * NW]
            s0_ = ANG_t[:, 0:NW]
            r0 = ANG_t[:, NW:2 * NW]
            kr = ANG_t[:, 2 * NW:3 * NW]
            qv = ANG_t[:, 3 * NW:4 * NW]
            m1 = ANG_t[:, 4 * NW:5 * NW]
            m2 = ANG_t[:, 5 * NW:6 * NW]
            nc.vector.tensor_scalar_add(Bg, B1, 1e-12)
            nc.vector.tensor_mul(R2, Bg, Bg)
            nc.vector.tensor_mul(t1, B2, B2)
            nc.vector.tensor_add(R2, R2, t1)
            nc.scalar.sqrt(s0_, R2)
            nc.vector.reciprocal(r0, s0_)
            nc.vector.tensor_mul(t1, R2, r0)          # Newton: s1 = .5(s0+R2/s0)
            nc.vector.tensor_add(t1, t1, s0_)
            nc.vector.tensor_scalar_mul(Spl, t1, 0.5)
            nc.vector.reciprocal(r0, Spl)             # rinv
            nc.vector.tensor_mul(kr, Bg, r0)
            nc.vector.tensor_mul(qv, B2, r0)          # ki = -qv
            nc.vector.tensor_mul(m1, kr, A1)
            nc.vector.tensor_mul(m2, qv, A2)
            nc.vector.tensor_sub(U00r, m1, m2)
            nc.vector.tensor_add(U11r, m1, m2)
            nc.vector.tensor_mul(m1, qv, A1)
            nc.vector.tensor_mul(m2, kr, A2)
            nc.vector.tensor_add(t1, m1, m2)
            nc.vector.tensor_scalar_mul(U00i, t1, -1.0)
            nc.vector.tensor_sub(U11i, m2, m1)
            nc.vector.tensor_mul(m1, kr, B1)
            nc.vector.tensor_mul(m2, qv, B2)
            nc.vector.tensor_sub(t1, m1, m2)
            nc.vector.tensor_scalar_mul(U01r, t1, -1.0)
            nc.vector.tensor_mul(m1, kr, B2)
            nc.vector.tensor_mul(m2, qv, B1)
            nc.vector.tensor_add(U01i, m1, m2)

            # --- state init (also initializes the cmulign dummy-in1 read
            # regions) + gates ----------------------------------------------
            nc.vector.memset(A_t[:], 0.0)

            tiles = {"A": A_t[:], "B": B_t[:], "T": T_t[:], "S": s64_t[:]}

            def ap(reg):
                return _ap(bass, tiles[reg[0]], reg)

            def scal(ref):
                if ref is None:
                    return 0.0
                pli, col = PLANE[ref[0]], ref[1]
                return PL2_t[:, pli * NW + col:pli * NW + col + 1]

            for op in sched:
                kind = op[0]
                if kind == "seed":
                    nc.vector.memset(T_t[:, 0:1], 1.0)
                    nc.vector.memset(T_t[:, 1:2], 0.0)
                elif kind == "cmulign":
                    _, dst, src, c0, c1 = op
                    _check_pair_eligible(dst)
                    _check_pair_eligible(src)
                    n = _nelem(src)
                    dummy = ("A", 0, ((1, n),))
                    emit_pair("cmulign", ap(dst), ap(src), ap(dummy),
                              scal(c0), scal(c1))
                elif kind == "cmulacc":
                    _, dst, src, acc, c0, c1 = op
                    _check_pair_eligible(dst)
                    _check_pair_eligible(src)
                    _check_pair_eligible(acc)
                    assert len(acc[2]) == 1
                    emit_pair("cmulacc", ap(dst), ap(src), ap(acc),
                              scal(c0), scal(c1))
                elif kind == "tsmul":
                    _, dst, srcr, sc = op
                    _check_pair_eligible(dst)
                    _check_pair_eligible(srcr)
                    nc.vector.tensor_scalar_mul(ap(dst), ap(srcr), scal(sc))
                elif kind == "sqsum":
                    _, dst, s0, s1 = op
                    nc.vector._custom_dve(
                        cops["sqsum"], out=ap(dst), in0=ap(s0), in1=ap(s1))
                elif kind == "red":
                    _, dst, src = op
                    nc.vector.tensor_reduce(
                        ap(dst), ap(src), axis=mybir.AxisListType.X,
                        op=mybir.AluOpType.add)
                else:
                    raise ValueError(kind)

            for w in range(NA):
                nc.vector.tensor_mul(
                    r64_t[:], s64_t[:], sg_t[:, w * 64:(w + 1) * 64])
                nc.vector.tensor_reduce(
                    out6_t[:, w:w + 1], r64_t[:],
                    axis=mybir.AxisListType.X, op=mybir.AluOpType.add)
            nc.sync.dma_start(out_d[tno * PT:(tno + 1) * PT, :], out6_t[:])

    # stamp perf_max=1 post-Tile (scheduling rebuilds instructions)
    names = set(pm_names)
    n_pm = 0
    for fn in nc.m.functions:
        for blk in fn.blocks:
            for inst in blk.instructions:
                if type(inst).__name__ == "InstCustomDveAnt" and inst.name in names:
                    inst.perf_max = 1
                    n_pm += 1
    assert n_pm == len(names), (n_pm, len(names))
    nc.compile()
    return nc


# ------------------------------------------------------------------- runner

_NC_CACHE = None
_RUNNER = None


class _Result:
    exec_time_ns = None


class Runner:
    """Persistent jitted SPMD executor (cached shard_map closure)."""

    def __init__(self, nc, n_cores=NCORES):
        import jax
        from jax.sharding import Mesh, PartitionSpec
        from jax.experimental.shard_map import shard_map
        from concourse import bass2jax
        import concourse.mybir as mybir

        bass2jax.install_neuronx_cc_hook()
        self.nc = nc
        self.n_cores = n_cores
        part_name = nc.partition_id_tensor.name if nc.partition_id_tensor else None
        in_names, out_names, out_avals, self.zero_shapes = [], [], [], []
        for alloc in nc.m.functions[0].allocations:
            if not isinstance(alloc, mybir.MemoryLocationSet):
                continue
            name = alloc.memorylocations[0].name
            if alloc.kind == "ExternalInput":
                if name != part_name:
                    in_names.append(name)
            elif alloc.kind == "ExternalOutput":
                out_names.append(name)
                shape = tuple(alloc.tensor_shape)
                dtype = mybir.dt.np(alloc.dtype)
                out_avals.append(jax.core.ShapedArray(shape, dtype))
                self.zero_shapes.append((shape, dtype))
        self.in_names = list(in_names)
        self.out_names = list(out_names)
        n_params = len(in_names)
        n_outs = len(out_names)
        all_in = list(in_names) + list(out_names)
        if part_name is not None:
            all_in.append(part_name)
        donate = tuple(range(n_params, n_params + n_outs))

        def _body(*args):
            operands = list(args)
            if nc.partition_id_tensor is not None:
                operands.append(bass2jax.partition_id_tensor())
            outs = bass2jax._bass_exec_p.bind(
                *operands,
                out_avals=tuple(out_avals),
                in_names=tuple(all_in),
                out_names=tuple(out_names),
                lowering_input_output_aliases=(),
                sim_require_finite=True,
                sim_require_nnan=True,
                nc=nc,
            )
            return tuple(outs)

        devices = jax.devices()[:n_cores]
        mesh = Mesh(np.asarray(devices), ("core",))
        in_specs = (PartitionSpec("core"),) * (n_params + n_outs)
        out_specs = (PartitionSpec("core"),) * n_outs
        self.fn = jax.jit(
            shard_map(_body, mesh=mesh, in_specs=in_specs,
                      out_specs=out_specs, check_rep=False),
            donate_argnums=donate,
            keep_unused=True,
        )

    def _sharding(self):
        import jax
        from jax.sharding import Mesh, NamedSharding, PartitionSpec
        if getattr(self, "_shard", None) is None:
            mesh = Mesh(np.asarray(jax.devices()[:self.n_cores]), ("core",))
            self._shard = NamedSharding(mesh, PartitionSpec("core"))
        return self._shard

    def _stage_zeros(self):
        # donated output buffers, staged on device between calls so the
        # timed call does no host->device zero transfer
        import jax
        sh = self._sharding()
        self._zeros = [
            jax.device_put(np.zeros((self.n_cores * s[0], *s[1:]), d), sh)
            for s, d in self.zero_shapes
        ]

    def put_input(self, arr):
        import jax
        return jax.device_put(arr, self._sharding())

    def __call__(self, concat_in):
        if getattr(self, "_zeros", None) is None:
            self._stage_zeros()
        zeros = self._zeros
        self._zeros = None
        out_arrs = self.fn(*concat_in, *zeros)
        res = [np.asarray(o) for o in out_arrs]
        self._stage_zeros()  # async restage for the next call
        return res


def run_cores(x_full, aux):
    global _NC_CACHE, _RUNNER
    if _NC_CACHE is None:
        _NC_CACHE = build_bass()
    nc = _NC_CACHE
    xs = np.asarray(x_full, np.float32).reshape(NCORES, BPC, OBS)
    aux_rows = np.asarray(aux, np.float32).reshape(1, NAUX // OBS, OBS)
    xa = np.concatenate(
        [xs, np.broadcast_to(aux_rows, (NCORES, NAUX // OBS, OBS))], axis=1)
    xa = np.ascontiguousarray(xa.reshape(NCORES * (BPC + NAUX // OBS), OBS))
    last_err = None
    for attempt in range(3):
        try:
            if _RUNNER is None:
                _RUNNER = Runner(nc)
            # fresh async device_put each call: overlaps the transfer with
            # dispatch. (Measured: REUSING a committed device array across
            # calls is ~25 ms slower through axon than re-putting fresh.)
            results = _RUNNER([_RUNNER.put_input(xa)])
            break
        except Exception as e:
            last_err = e
            if _RUNNER is not None:
                _RUNNER._zeros = None
            import time as _time
            _time.sleep(45 * (attempt + 1))
    else:
        raise last_err
    return results[0], _Result()


def kernel(x, input_scaling, weights, action_scale, action_bias):
    aux = make_aux(input_scaling, weights)
    s6, _ = run_cores(np.asarray(x, np.float32), aux)
    return s6 * np.asarray(action_scale, np.float32) + np.asarray(
        action_bias, np.float32)
